# revision 20
# baseline (speedup 1.0000x reference)
"""Distance-aware multi-head attention on 8 trn2 NeuronCores.

Sharding: pure data-parallel over batch (B=8 -> one batch element per core,
no collectives).  Per core, the dominant cost is streaming the 67MB (f32)
dist_encoding slice; we cast to bf16 on the host and pre-permute it so the
device consumes it as pair-packed PE stationary tiles at full DMA and
weight-load rate.

Math per core (batch b):
  Q^T_h [64,512q]  = (Wq/8)^T x^T          (scale folded into Wq)
  K^T_h [64,512q]  = Wk^T x^T
  V_kt  [128k,512(h,d)] = x W v
  biasT[k,q,h]     = pair-packed dist tiles (stationary) @ blockdiag(Wd,Wd)
  S(h,kt)[128k,512q] = K^T_h(kt)^T Q^T_h  + biasT(strided gather)
  expT = Exp(S + madd_k + bd_h)            (ACT per-partition bias = mask fill)
  AV(h)[65,512q]   = sum_kt [V_h | 1]^T expT   (row 64 = softmax denominator)
  nm[h,q] = mask_q[q] / denom[h,q]; broadcast via row-select matmul
  attnOT[hd,q] = AV[0:64] * nm ;  out[q,:] = attnOT^T Wo (+bo) (*mask_q via nm)
"""

import os
import sys
import threading

for p in ("/opt/trn_rl_repo/concourse", "/opt/trn_rl_repo", "/opt/pypackages"):
    if p not in sys.path:
        sys.path.insert(0, p)

import numpy as np
import ml_dtypes

BF16 = ml_dtypes.bfloat16

B = 8
N = 512          # sequence length
H = 512          # hidden
NH = 8           # heads
D = 64           # head dim
DD = 64          # dist dim
SCALE = float(np.sqrt(D))
NKH = 2          # k halves (256 each)
NQP = N // 2     # 256 q-pairs
NKW = 256        # k within half
NKT = 4          # 128-wide k tiles
NQB = 4          # 128-wide q tiles
QG = 32          # q-pairs per dist DMA chunk
NQG = NQP // QG  # 8 chunks per k-half

_lock = threading.Lock()
_cache = {}


def _build_bass():
    import concourse.bass as bass
    import concourse.mybir as mybir
    import concourse.tile as tile

    f32 = mybir.dt.float32
    bf16 = mybir.dt.bfloat16
    Exp = mybir.ActivationFunctionType.Exp
    add_op = mybir.AluOpType.add
    mult_op = mybir.AluOpType.mult

    nc = bass.Bass()

    dist_d = nc.dram_tensor("distH", [NKH, 128, NQP, NKW], bf16, kind="ExternalInput")
    bigw_d = nc.dram_tensor("bigw", [128, 20 * H], bf16, kind="ExternalInput")
    wdd_d = nc.dram_tensor("wdd", [128, 16], bf16, kind="ExternalInput")
    madh_d = nc.dram_tensor("madh", [128, NH * NKT], f32, kind="ExternalInput")
    mqrow_d = nc.dram_tensor("mqrow", [1, N], f32, kind="ExternalInput")
    out_d = nc.dram_tensor("out", [N, H], f32, kind="ExternalOutput")

    with tile.TileContext(nc) as tc:
        with (
            tc.tile_pool(name="wpool", bufs=1) as wpool,
            tc.tile_pool(name="dpool", bufs=2) as dpool,
            tc.tile_pool(name="spool", bufs=1) as spool,
            tc.tile_pool(name="ps", bufs=8, space="PSUM") as ps,
        ):
            # ---- load weights / constants ----
            bigw = wpool.tile([128, 20 * H], bf16, tag="bigw", name="bigw")
            nc.sync.dma_start(bigw[:], bigw_d[:])

            def wslice(i):
                return [bigw[:, (4 * i + c) * H:(4 * i + c + 1) * H] for c in range(4)]

            xT, wq, wk, wv, wo = (wslice(i) for i in range(5))

            wdd_raw = wpool.tile([128, 16], bf16, tag="wddr", name="wdd_raw")
            nc.sync.dma_start(wdd_raw[:], wdd_d[:])
            wdd = wpool.tile([128, 16], bf16, tag="wdd", name="wdd_t")
            nc.vector.tensor_copy(wdd[:], wdd_raw[:])
            madh_raw = wpool.tile([128, NH * NKT], f32, tag="madhr", name="madh_raw")
            nc.sync.dma_start(madh_raw[:], madh_d[:])
            madh = wpool.tile([128, NH * NKT], f32, tag="madh", name="madh_t")
            nc.vector.tensor_copy(madh[:], madh_raw[:])
            mqrow = wpool.tile([1, N], f32, tag="mqrow", name="mqrow_t")
            nc.sync.dma_start(mqrow[:], mqrow_d[:])
            ones64 = wpool.tile([1, 64], f32, tag="ones64", name="ones64")
            nc.vector.memset(ones64[:], 1.0)
            absorb = wpool.tile([1, 64], bf16, tag="absorb", name="absorb")

            # ---- phase A: projections ----
            QT = []  # per head [64, N] bf16
            KT = []
            for dst, w in ((QT, wq), (KT, wk)):
                for hp in range(4):
                    acc = ps.tile([128, N], f32, tag="ps", name="psA")
                    for c in range(4):
                        nc.tensor.matmul(
                            acc[:], w[c][:, hp * 128:(hp + 1) * 128], xT[c],
                            start=(c == 0), stop=(c == 3),
                        )
                    for par in range(2):
                        t = spool.tile([64, N], bf16, tag=f"qk_{w is wq}_{len(dst)}", name=f"qk_{w is wq}_{len(dst)}")
                        nc.vector.tensor_copy(t[:], acc[par * 64:(par + 1) * 64, :])
                        dst.append(t)

            Vext = []  # [kt][h] -> [128, 65] bf16 (V_h | ones)
            for kt in range(NKT):
                acc = ps.tile([128, H], f32, tag="ps", name="psB")
                for c in range(4):
                    nc.tensor.matmul(
                        acc[:], xT[c][:, kt * 128:(kt + 1) * 128], wv[c],
                        start=(c == 0), stop=(c == 3),
                    )
                row = []
                for h in range(NH):
                    t = spool.tile([128, D + 1], bf16, tag=f"vx{kt}_{h}", name=f"vx{kt}_{h}")
                    nc.vector.tensor_copy(t[:, 0:D], acc[:, h * D:(h + 1) * D])
                    nc.vector.memset(t[:, D:D + 1], 1.0)
                    row.append(t)
                Vext.append(row)

            # ---- phase B/C interleaved over k-halves ----
            bigT = [
                spool.tile([128, NQP * 16], bf16, tag=f"bigT{kt}", name=f"bigT{kt}")
                for kt in range(NKT)
            ]
            expT = [[None] * NKT for _ in range(NH)]

            def phaseB(kh):
                for qg in range(NQG):
                    dt_ = dpool.tile([128, QG * NKW], bf16, tag="dist", name="dist_t")
                    src = dist_d[kh, :, qg * QG:(qg + 1) * QG, :]
                    nc.scalar.dma_start(dt_[:], src.rearrange("p q k -> p (q k)"))
                    for kwh in range(2):
                        kt = 2 * kh + kwh
                        bank = ps.tile([128, 512], f32, tag="ps", name="psC")
                        for ql in range(QG):
                            lhsT = dt_[:, ql * NKW + kwh * 128: ql * NKW + kwh * 128 + 128]
                            nc.tensor.matmul(
                                bank[:, ql * 16:(ql + 1) * 16], lhsT, wdd[:],
                                start=True, stop=True,
                            )
                        nc.vector.tensor_copy(
                            bigT[kt][:, qg * 512:(qg + 1) * 512], bank[:]
                        )
                    # dep-absorber: ACT reads cols overlapping every LDW slice
                    # and writes back into the (dead) tile, so the next DMA to
                    # this slot orders behind it and inherits only an ACT wait
                    nc.scalar.copy(dt_[0:1, 0:64], dt_[0:1, 1::128])

            attnOT = [
                spool.tile([128, N], bf16, tag=f"aot{p}", name=f"aot{p}") for p in range(4)
            ]

            def score_exp(h, kt, tag):
                S = ps.tile([128, N], f32, tag="ps", name="psA")
                nc.tensor.matmul(
                    S[:], KT[h][:, kt * 128:(kt + 1) * 128], QT[h][:],
                    start=True, stop=True,
                )
                nc.vector.tensor_tensor(S[:], S[:], bigT[kt][:, h::8], add_op)
                e = spool.tile([128, N], bf16, tag=tag, name=f"expT{h}_{kt}",
                               bufs=6 if tag == "expT23" else 1)
                idx = h * NKT + kt
                nc.scalar.activation(
                    e[:], S[:], Exp, bias=madh[:, idx:idx + 1], scale=1.0
                )
                expT[h][kt] = e

            def attn_v(h):
                AV = ps.tile([65, N], f32, tag="ps", name="psAV")
                for kt in range(NKT):
                    nc.tensor.matmul(
                        AV[:], Vext[kt][h][:], expT[h][kt][:],
                        start=(kt == 0), stop=(kt == NKT - 1),
                    )
                rsrow = spool.tile([1, N], f32, tag="rsr", name=f"rsr{h}", bufs=3)
                nc.vector.tensor_copy(rsrow[:], AV[64:65, :])
                rrec = spool.tile([1, N], f32, tag="rrc", name=f"rrc{h}", bufs=3)
                nc.vector.reciprocal_approx_fast(rrec[:], rsrow[:])
                nm = spool.tile([1, N], f32, tag="nm", name=f"nm{h}", bufs=3)
                nc.vector.tensor_tensor(nm[:], rrec[:], mqrow[:], mult_op)
                nmb = ps.tile([64, N], f32, tag="ps", name="psNMB")
                nc.tensor.matmul(nmb[:], ones64[:], nm[:], start=True, stop=True)
                nmb_sb = spool.tile([64, N], f32, tag="nmbsb", name=f"nmbsb{h}",
                                    bufs=3)
                nc.vector.tensor_copy(nmb_sb[:], nmb[:])
                dst = attnOT[h // 2][(h % 2) * 64:(h % 2) * 64 + 64, :]
                nc.vector.tensor_tensor(dst, AV[0:64, :], nmb_sb[:], mult_op)

            phaseB(0)
            for h in range(NH):
                for kt in (0, 1):
                    score_exp(h, kt, f"expT{h}_{kt}")
            phaseB(1)
            for h in range(NH):
                for kt in (2, 3):
                    score_exp(h, kt, "expT23")
                attn_v(h)

            # ---- output projection ----
            for qb in range(NQB):
                O = ps.tile([128, H], f32, tag="ps", name="psB")
                for c in range(4):
                    nc.tensor.matmul(
                        O[:], attnOT[c][:, qb * 128:(qb + 1) * 128], wo[c],
                        start=(c == 0), stop=(c == 3),
                    )
                ot = spool.tile([128, H], f32, tag="osb", name="osb", bufs=2)
                nc.scalar.copy(ot[:], O[:])
                nc.scalar.dma_start(out_d[qb * 128:(qb + 1) * 128, :], ot[:])

    _strip_self_waits(nc)
    _fit_sync_limits(nc)
    from concourse.library_overlay import lower_extended_insts
    lower_extended_insts(nc)
    return nc


def _strip_self_waits(nc):
    """Remove same-engine semaphore waits (vacuous: engines execute in
    program order) so instructions fit walrus' per-instruction sync-command
    limits."""
    import concourse.mybir as mybir
    eng_sem = {
        mybir.EngineType.PE: "PE_",
        mybir.EngineType.DVE: "DVE_",
        mybir.EngineType.Activation: "Activation_",
        mybir.EngineType.SP: "SP_",
        mybir.EngineType.Pool: "Pool_",
    }
    for blk in nc.m.functions[0].blocks:
        for i in blk.instructions:
            si = i.sync_info
            if not si or not si.on_wait:
                continue
            eng = getattr(i, "engine", None)
            pref = eng_sem.get(eng)
            if pref is not None:
                kept = [w for w in si.on_wait if not w.ant_name.startswith(pref)]
                if len(kept) != len(si.on_wait):
                    si.on_wait = kept
            # dist-stream DMAs: a PE wait (WAR vs this slot's readers)
            # transitively implies the predecessor DMA completed, making a
            # coexisting cross-lane DMAHW wait redundant.
            if type(i).__name__ == "InstDMACopy" and any(
                "dist_t" in getattr(o, "memref", "") for o in i.outs
            ):
                w = si.on_wait
                if len(w) > 1 and any(x.ant_name.startswith("PE_") for x in w):
                    si.on_wait = [
                        x for x in w if not x.ant_name.startswith("DMAHW")
                    ]


_FITTABLE = {
    "InstMatmult", "InstLdweights", "InstActivation", "InstTensorTensor",
    "InstTensorCopy", "InstTensorScalarPtr", "InstCustomDveAnt",
    "InstMemset", "InstReciprocal", "InstDMACopy", "InstTensorReduce",
    "InstDrain",
}


def _fit_sync_limits(nc):
    """Walrus' 64B instruction encodings fit 3 sync slots; a wait costs 2,
    an update 1 — so at most ONE wait per instruction.  Hoist excess waits
    onto same-engine NOPs injected just before the instruction — the NX
    sequencer executes the NOP's waits first, which is semantically
    identical."""
    import concourse.mybir as mybir

    for blk in nc.m.functions[0].blocks:
        il = blk.instructions
        out = []
        for inst in il:
            si = inst.sync_info
            if (
                type(inst).__name__ not in _FITTABLE
                or si is None
                or not si.on_wait
            ):
                out.append(inst)
                continue
            waits = list(si.on_wait)
            if len(waits) <= 1:
                out.append(inst)
                continue
            excess, kept = waits[:-1], waits[-1:]
            for j, w in enumerate(excess):
                nop = mybir.InstNoOp(
                    name=f"{inst.name}-hw{j}",
                    engine=inst.engine,
                    ins=[],
                    outs=[],
                    sync_info=mybir.SyncInfo(on_wait=[w], on_update=[]),
                )
                out.append(nop)
            si.on_wait = kept
            out.append(inst)
        il[:] = out


def _get_bass():
    with _lock:
        if "nc" not in _cache:
            _cache["nc"] = _build_bass()
        return _cache["nc"]


def _prep_core(b, x, dist, mask, wq_s, wk, wv, wo, wdd, bd):
    """Build the per-core input map for batch element b."""
    xT = np.ascontiguousarray(x[b].T).astype(BF16)
    d = dist[b].reshape(NQP, 2, NKH, NKW, DD)
    distH = np.ascontiguousarray(d.transpose(2, 1, 4, 0, 3)).reshape(
        NKH, 128, NQP, NKW
    ).astype(BF16)
    mk = mask[b].astype(np.float32)
    madd = np.where(mk > 0.5, 0.0, -1e9).astype(np.float32)
    madh = np.empty((128, NH * NKT), np.float32)
    for h in range(NH):
        for kt in range(NKT):
            madh[:, h * NKT + kt] = madd[kt * 128:(kt + 1) * 128] + float(bd[h])
    return {
        "distH": distH,
        "xT": xT,
        "madh": madh,
        "mqrow": mk.reshape(1, N).copy(),
    }


def kernel(x, dist_encoding, mask, Wq, bq, Wk, bk, Wv, bv, Wo, bo, Wd, bd,
           trace=False):
    from concourse.bass_utils import run_bass_kernel_spmd

    x = np.asarray(x, dtype=np.float32)
    dist = np.asarray(dist_encoding, dtype=np.float32)
    mask = np.asarray(mask)
    Wq = np.asarray(Wq, np.float32); Wk = np.asarray(Wk, np.float32)
    Wv = np.asarray(Wv, np.float32); Wo = np.asarray(Wo, np.float32)
    Wd = np.asarray(Wd, np.float32)
    bq = np.asarray(bq, np.float32); bk = np.asarray(bk, np.float32)
    bv = np.asarray(bv, np.float32); bo = np.asarray(bo, np.float32)
    bd = np.asarray(bd, np.float32)
    assert not (np.any(bq) or np.any(bk) or np.any(bv) or np.any(bo)), \
        "nonzero qkvo biases not wired"

    # shared (replicated) weights
    wq_s = np.ascontiguousarray(Wq / SCALE).astype(BF16)
    wk_b = np.ascontiguousarray(Wk).astype(BF16)
    wv_b = np.ascontiguousarray(Wv).astype(BF16)
    wo_b = np.ascontiguousarray(Wo).astype(BF16)
    wdd = np.zeros((128, 16), np.float32)
    wdd[0:64, 0:8] = Wd
    wdd[64:128, 8:16] = Wd
    wdd = wdd.astype(BF16)

    from concurrent.futures import ThreadPoolExecutor
    with ThreadPoolExecutor(max_workers=8) as ex:
        percore = list(ex.map(
            lambda b: _prep_core(b, x, dist, mask, wq_s, wk_b, wv_b, wo_b,
                                 wdd, bd),
            range(B),
        ))
    in_maps = []
    for b in range(B):
        m = dict(percore[b])
        xT_b = m.pop("xT")
        m["bigw"] = np.ascontiguousarray(np.concatenate([
            xT_b.reshape(4, 128, H),
            wq_s.reshape(4, 128, H), wk_b.reshape(4, 128, H),
            wv_b.reshape(4, 128, H), wo_b.reshape(4, 128, H),
        ]).reshape(20, 128, H).transpose(1, 0, 2).reshape(128, 20 * H))
        m["wdd"] = wdd
        in_maps.append(m)

    nc = _get_bass()
    kernel.last_in_maps = in_maps
    res = run_bass_kernel_spmd(nc, in_maps, list(range(B)), trace=False)
    out = np.stack([res.results[b]["out"] for b in range(B)]).astype(np.float32)
    if trace:
        kernel.last_exec_time_ns = res.exec_time_ns
        kernel.last_results = res
    return out


def bench_exec_ns(in_maps=None, iters=12):
    """Estimate per-execution HW time: steady-state wall time of the jitted
    SPMD kernel with device-resident inputs, minus bare dispatch overhead."""
    import time
    import jax
    import jax.numpy as jnp
    from jax.sharding import Mesh, PartitionSpec
    from jax.experimental.shard_map import shard_map
    import concourse.bass2jax as b2j
    import concourse.mybir as mybir

    nc = _get_bass()
    if in_maps is None:
        in_maps = kernel.last_in_maps
    n_cores = len(in_maps)

    partition_name = nc.partition_id_tensor.name if nc.partition_id_tensor else None
    in_names, out_names, out_avals, zero_outs = [], [], [], []
    for alloc in nc.m.functions[0].allocations:
        if not isinstance(alloc, mybir.MemoryLocationSet):
            continue
        name = alloc.memorylocations[0].name
        if alloc.kind == "ExternalInput":
            if name != partition_name:
                in_names.append(name)
        elif alloc.kind == "ExternalOutput":
            out_names.append(name)
            shape = tuple(alloc.tensor_shape)
            dtype = mybir.dt.np(alloc.dtype)
            out_avals.append(jax.core.ShapedArray(shape, dtype))
            zero_outs.append(np.zeros(shape, dtype))
    n_params = len(in_names)
    n_outs = len(out_avals)
    all_in_names = list(in_names) + out_names
    if partition_name is not None:
        all_in_names.append(partition_name)

    def _body(*args):
        operands = list(args)
        if partition_name is not None:
            operands.append(b2j.partition_id_tensor())
        outs = b2j._bass_exec_p.bind(
            *operands,
            out_avals=tuple(out_avals),
            in_names=tuple(all_in_names),
            out_names=tuple(out_names),
            lowering_input_output_aliases=(),
            sim_require_finite=True,
            sim_require_nnan=True,
            nc=nc,
        )
        return tuple(outs)

    devices = jax.devices()[:n_cores]
    mesh = Mesh(np.asarray(devices), ("core",))
    in_specs = (PartitionSpec("core"),) * (n_params + n_outs)
    out_specs = (PartitionSpec("core"),) * n_outs
    fn = jax.jit(
        shard_map(_body, mesh=mesh, in_specs=in_specs, out_specs=out_specs,
                  check_rep=False),
        keep_unused=True,
    )
    from jax.sharding import NamedSharding
    shardng = NamedSharding(mesh, PartitionSpec("core"))
    concat_in = [
        jax.device_put(
            np.concatenate([np.asarray(in_maps[c][in_names[i]])
                            for c in range(n_cores)], axis=0), shardng)
        for i in range(n_params)
    ]
    concat_zeros = [
        jax.device_put(
            np.zeros((n_cores * z.shape[0], *z.shape[1:]), z.dtype), shardng)
        for z in zero_outs
    ]
    # warmup (compile)
    out = fn(*concat_in, *concat_zeros)
    jax.block_until_ready(out)
    times = []
    for _ in range(iters):
        t0 = time.perf_counter()
        out = fn(*concat_in, *concat_zeros)
        jax.block_until_ready(out)
        times.append(time.perf_counter() - t0)
    t_kernel = min(times)

    # dispatch overhead baseline: trivial sharded op
    tiny = jax.device_put(np.zeros((n_cores, 8), np.float32), shardng)
    triv = jax.jit(shard_map(lambda x: x + 1.0, mesh=mesh,
                             in_specs=PartitionSpec("core"),
                             out_specs=PartitionSpec("core"), check_rep=False))
    o = triv(tiny); jax.block_until_ready(o)
    tt = []
    for _ in range(iters):
        t0 = time.perf_counter()
        o = triv(tiny)
        jax.block_until_ready(o)
        tt.append(time.perf_counter() - t0)
    t_triv = min(tt)
    return {
        "kernel_wall_ns": t_kernel * 1e9,
        "dispatch_ns": t_triv * 1e9,
        "exec_est_ns": (t_kernel - t_triv) * 1e9,
    }
